# revision 4
# baseline (speedup 1.0000x reference)
"""GCN (2-layer) on 8 Trainium2 NeuronCores.

Strategy (graph/data parallel, per the node-range sharding hint):
- Nodes are sharded by range (25k per core); edges live on the core that
  owns their *destination* node; tiny weights are replicated.
- All irregular graph routing is converted on the host into REGULAR
  device-side layouts:
    * scatter side: destination nodes are bucketed by in-degree class
      j=ceil(d/8); each node gets exactly 8j message slots, so segment-sum
      becomes plain `tensor_reduce` over the innermost axis.
    * gather side: source rows are bucketed by multiplicity class m
      (# edges this core pulls from the row); the per-core permuted table
      is expanded by stride-0 broadcast copies [P,b,f] -> [P,b,f,m].
- The per-edge routing between gather order and scatter order (the
  "all-to-all on gathered messages") is staged through the host between
  device programs; every floating-point operation on values happens on
  device.

Five small SPMD programs: deg/dis -> expand L1 -> reduce L1 + MLP ->
expand L2 -> reduce L2.
"""
import sys

sys.path.insert(0, "/opt/trn_rl_repo")

import numpy as np

import bass_rust
from concourse import bass, mybir
from concourse.bass_utils import run_bass_kernel_spmd
import concourse.tile as tile

import os as _os

PROGRAM_TIMES_NS = []   # (name, exec_time_ns) per device program of last kernel() call


def _enable_tracing():
    import types
    import antenv
    if 'antenv.axon_hooks' in sys.modules:
        return True
    try:
        from trn_agent_boot.trn_boot import _ntff_profile_via_ctypes
        hook = _ntff_profile_via_ctypes('/opt/axon/libaxon_pjrt.so')
    except Exception:
        return False
    mod = types.ModuleType('antenv.axon_hooks')
    mod.get_axon_ntff_profile_hook = lambda: hook
    mod.set_axon_ntff_profile_hook = lambda h: None
    sys.modules['antenv.axon_hooks'] = mod
    antenv.axon_hooks = mod
    import concourse.bass_utils as _bu
    _bu.upload_artifacts = lambda tmpdir: f"local://{tmpdir}"
    return True


def _run(nc, in_maps, name):
    trace = bool(_os.environ.get('GCN_TRACE')) and _enable_tracing()
    r = run_bass_kernel_spmd(nc, in_maps, core_ids=CORE_IDS, trace=trace)
    if trace:
        PROGRAM_TIMES_NS.append((name, r.exec_time_ns))
    return r.results

S = 8
N = 200000
NS = N // S
P = 128
F1 = 4
F2 = 7
CORE_IDS = list(range(S))
FP = mybir.dt.float32
MUL = mybir.AluOpType.mult
ADD = mybir.AluOpType.add


def _ceil(a, b):
    return -(-a // b)


# --------------------------------------------------------------------------
# walrus on this toolchain accepts at most ONE sync-wait per instruction;
# Tile emits several at DAG joins / kernel-tail drain. Hoist excess waits
# onto fresh same-engine NoOps inserted immediately before the violator.
def legalize_waits(nc):
    nop_idx = 0
    for f in nc.m.functions:
        for bb in f.blocks:
            il = bb.instructions
            if not any(
                inst.sync_info is not None
                and len(inst.sync_info.on_wait or []) > 1
                for inst in il
            ):
                continue
            new_il = []
            for inst in il:
                si = inst.sync_info
                w = list(si.on_wait or []) if si is not None else []
                if len(w) > 1:
                    for extra in w[:-1]:
                        nop = mybir.InstNoOp(
                            name=f"I-waitsplit-{nop_idx}", ins=[], outs=[]
                        )
                        nop_idx += 1
                        nop.engine = inst.engine
                        nop.sync_info = bass_rust.SyncInfo(
                            on_wait=[extra], on_update=[]
                        )
                        new_il.append(nop)
                    si.on_wait = [w[-1]]
                new_il.append(inst)
            bb.instructions = new_il


# --------------------------------------------------------------------------
# host-side structure building
class _O:
    pass


def build_structs(row, col, ew):
    row = row.astype(np.int64)
    col = col.astype(np.int64)
    cores = []
    for c in range(S):
        cs = _O()
        m = (col // NS) == c
        cs.erow = row[m]
        cs.ecol = (col[m] - c * NS).astype(np.int64)
        cs.eew = ew[m].astype(np.float32)
        cores.append(cs)

    for cs in cores:
        d = np.bincount(cs.ecol, minlength=NS)
        cs.indeg = d
        cs.jcls = np.maximum(1, _ceil(np.maximum(d, 1), 8))
    jmax = max(int(cs.jcls.max()) for cs in cores)
    nj = np.zeros(jmax + 1, np.int64)
    for j in range(1, jmax + 1):
        njc = max(int((cs.jcls == j).sum()) for cs in cores)
        nj[j] = _ceil(max(njc, 1), P) * P
    for cs in cores:
        nodepos = np.full(NS, -1, np.int64)
        pos = 0
        for j in range(1, jmax + 1):
            nodes = np.nonzero(cs.jcls == j)[0]
            nodepos[nodes] = pos + np.arange(len(nodes))
            pos += nj[j]
        cs.nodepos = nodepos
    ntot = int(nj[1:].sum())

    for cs in cores:
        rows_used, inv, cnt = np.unique(
            cs.erow, return_inverse=True, return_counts=True
        )
        cs.g_rows = rows_used
        cs.g_cnt = cnt
        cs.g_inv = inv
    mmax = max(int(cs.g_cnt.max()) for cs in cores)
    tm = np.zeros(mmax + 1, np.int64)
    for mcl in range(1, mmax + 1):
        tmc = max(int((cs.g_cnt == mcl).sum()) for cs in cores)
        tm[mcl] = _ceil(max(tmc, 1), P) * P
    for cs in cores:
        tabrows = np.full(int(tm[1:].sum()), -1, np.int64)
        pos_of_unique = np.empty(len(cs.g_rows), np.int64)
        pos = 0
        for mcl in range(1, mmax + 1):
            sel = cs.g_cnt == mcl
            rr = cs.g_rows[sel]
            tabrows[pos : pos + len(rr)] = rr
            pos_of_unique[sel] = pos - 0 + np.arange(len(rr))
            pos += tm[mcl]
        cs.tabrows = tabrows
        cs.g_tabpos = pos_of_unique[cs.g_inv]
    rtot = int(tm[1:].sum())

    st = _O()
    st.jmax, st.nj, st.ntot = jmax, nj, ntot
    st.mmax, st.tm, st.rtot = mmax, tm, rtot
    st.cores = cores
    base_tab = np.zeros(mmax + 2, np.int64)
    base_tab[1:] = np.cumsum(tm)[: mmax + 1]
    st.base_tab = base_tab

    for cs in cores:
        mm = cs.g_cnt[cs.g_inv]
        ord_ = np.argsort(cs.g_inv, kind="stable")
        inv_sorted = cs.g_inv[ord_]
        first = np.r_[True, inv_sorted[1:] != inv_sorted[:-1]]
        idx_of_first = np.maximum.accumulate(
            np.where(first, np.arange(len(ord_)), 0)
        )
        occ = np.empty(len(cs.erow), np.int64)
        occ[ord_] = np.arange(len(ord_)) - idx_of_first
        cs.g_m = mm
        cs.g_occ = occ
    return st


def gather_flat_index(st, cs, f):
    mm = cs.g_m
    q_local = cs.g_tabpos - st.base_tab[mm]
    tbm = st.tm[mm] // P
    p = q_local // tbm
    b = q_local % tbm
    base_free = np.zeros(st.mmax + 1, np.int64)
    acc = 0
    for mcl in range(1, st.mmax + 1):
        base_free[mcl] = acc
        acc += (st.tm[mcl] // P) * f * mcl
    gfree = base_free[mm] + b * (f * mm) + cs.g_occ
    return p, gfree, acc


def scatter_flat_index(st, cs, f):
    nodes = cs.ecol
    j = cs.jcls[nodes]
    q = cs.nodepos[nodes]
    base_node = np.zeros(st.jmax + 1, np.int64)
    accn = 0
    for jj in range(1, st.jmax + 1):
        base_node[jj] = accn
        accn += st.nj[jj]
    q_local = q - base_node[j]
    nbj = st.nj[j] // P
    p = q_local // nbj
    b = q_local % nbj
    base_free = np.zeros(st.jmax + 1, np.int64)
    acc = 0
    for jj in range(1, st.jmax + 1):
        base_free[jj] = acc
        acc += (st.nj[jj] // P) * f * 8 * jj
    ord_ = np.argsort(nodes, kind="stable")
    ns = nodes[ord_]
    first = np.r_[True, ns[1:] != ns[:-1]]
    idx_of_first = np.maximum.accumulate(np.where(first, np.arange(len(ord_)), 0))
    k = np.empty(len(nodes), np.int64)
    k[ord_] = np.arange(len(nodes)) - idx_of_first
    sfree = base_free[j] + b * (f * 8 * j) + k
    return p, sfree, acc


def own_perm(st, cs):
    """per local node: (partition, block) in the scatter/agg [P, ntot/P] grid"""
    base_node = np.zeros(st.jmax + 1, np.int64)
    accn = 0
    for jj in range(1, st.jmax + 1):
        base_node[jj] = accn
        accn += st.nj[jj]
    base_nb = np.zeros(st.jmax + 1, np.int64)
    accb = 0
    for jj in range(1, st.jmax + 1):
        base_nb[jj] = accb
        accb += st.nj[jj] // P
    j = cs.jcls
    ql = cs.nodepos - base_node[j]
    nbj = st.nj[j] // P
    return ql // nbj, base_nb[j] + ql % nbj


def tab_place(st):
    """table position q -> (p, block) in [P, rtot/P]."""
    pp = np.empty(st.rtot, np.int64)
    bb = np.empty(st.rtot, np.int64)
    accb = 0
    pos = 0
    for mcl in range(1, st.mmax + 1):
        tbm = st.tm[mcl] // P
        ql = np.arange(st.tm[mcl])
        pp[pos : pos + st.tm[mcl]] = ql // tbm
        bb[pos : pos + st.tm[mcl]] = accb + ql % tbm
        pos += st.tm[mcl]
        accb += tbm
    return pp, bb


# --------------------------------------------------------------------------
# device programs
_CHUNK = 6144  # free-size chunk budget (fp32 elems per partition) for streaming


def _chunks(total, step):
    out = []
    o = 0
    while o < total:
        out.append((o, min(step, total - o)))
        o += step
    return out


def build_PA(st):
    """ews [P, SF1] -> dis [P, ntot/P] ; deg = reduce + 1 ; dis = rsqrt."""
    nc = bass.Bass("TRN2", num_devices=S)
    SF1 = sum((int(st.nj[j]) // P) * 8 * j for j in range(1, st.jmax + 1))
    nb_all = st.ntot // P
    ews = nc.dram_tensor("ews", (P, SF1), FP, kind="ExternalInput")
    dis_o = nc.dram_tensor("dis", (P, nb_all), FP, kind="ExternalOutput")
    with tile.TileContext(nc) as tc:
        with tc.tile_pool(name="sb", bufs=2) as pool, tc.tile_pool(
            name="acc", bufs=1
        ) as apool:
            t_deg = apool.tile([P, nb_all], FP)
            accf = 0
            accb = 0
            for j in range(1, st.jmax + 1):
                nbj = int(st.nj[j]) // P
                L = 8 * j
                for b0, bl in _chunks(nbj, max(1, _CHUNK // L)):
                    t_in = pool.tile([P, bl * L], FP, tag="in")
                    nc.sync.dma_start(
                        out=t_in[:],
                        in_=ews[:, accf + b0 * L : accf + (b0 + bl) * L],
                    )
                    nc.vector.tensor_reduce(
                        out=t_deg[:, accb + b0 : accb + b0 + bl],
                        in_=t_in[:].rearrange("p (b l) -> p b l", l=L),
                        axis=mybir.AxisListType.X,
                        op=ADD,
                    )
                accf += nbj * L
                accb += nbj
            t_d1 = apool.tile([P, nb_all], FP)
            nc.vector.tensor_scalar_add(t_d1[:], t_deg[:], 1.0)
            t_sq = apool.tile([P, nb_all], FP)
            nc.scalar.sqrt(t_sq[:], t_d1[:])
            t_r = apool.tile([P, nb_all], FP)
            nc.vector.reciprocal(t_r[:], t_sq[:])
            # one Newton step: y <- y * (1.5 - 0.5 * d * y^2)
            t_y2 = apool.tile([P, nb_all], FP)
            nc.vector.tensor_tensor(t_y2[:], t_r[:], t_r[:], MUL)
            nc.vector.tensor_tensor(t_y2[:], t_y2[:], t_d1[:], MUL)
            nc.vector.tensor_scalar_mul(t_y2[:], t_y2[:], -0.5)
            nc.vector.tensor_scalar_add(t_y2[:], t_y2[:], 1.5)
            nc.vector.tensor_tensor(t_r[:], t_r[:], t_y2[:], MUL)
            nc.sync.dma_start(out=dis_o[:], in_=t_r[:])
    legalize_waits(nc)
    return nc


def build_expand(st, F, scale_dis):
    """x_tab [P, RB*F] (+ dis_tab [P, RB] if scale_dis), ewg [P, EWT]
    -> msgs_g [P, GF]."""
    nc = bass.Bass("TRN2", num_devices=S)
    RB = st.rtot // P
    EWT = sum((int(st.tm[m]) // P) * m for m in range(1, st.mmax + 1))
    GF = sum((int(st.tm[m]) // P) * F * m for m in range(1, st.mmax + 1))
    x_tab = nc.dram_tensor("x_tab", (P, RB * F), FP, kind="ExternalInput")
    ewg = nc.dram_tensor("ewg", (P, EWT), FP, kind="ExternalInput")
    if scale_dis:
        dis_tab = nc.dram_tensor("dis_tab", (P, RB), FP, kind="ExternalInput")
    msgs = nc.dram_tensor("msgs", (P, GF), FP, kind="ExternalOutput")
    with tile.TileContext(nc) as tc:
        with tc.tile_pool(name="tab", bufs=1) as tpool, tc.tile_pool(
            name="str", bufs=3
        ) as pool:
            t_tab = tpool.tile([P, RB * F], FP)
            nc.sync.dma_start(out=t_tab[:], in_=x_tab[:])
            if scale_dis:
                t_dis = tpool.tile([P, RB], FP)
                nc.sync.dma_start(out=t_dis[:], in_=dis_tab[:])
                nc.vector.tensor_tensor(
                    t_tab[:].rearrange("p (b f) -> p b f", f=F),
                    t_tab[:].rearrange("p (b f) -> p b f", f=F),
                    t_dis[:].unsqueeze(2).broadcast_to([P, RB, F]),
                    MUL,
                )
            t_ew = tpool.tile([P, EWT], FP)
            nc.sync.dma_start(out=t_ew[:], in_=ewg[:])
            accb = 0
            accw = 0
            accg = 0
            for m in range(1, st.mmax + 1):
                tbm = int(st.tm[m]) // P
                for b0, bl in _chunks(tbm, max(1, _CHUNK // (F * m))):
                    t_out = pool.tile([P, bl * F * m], FP, tag="out")
                    src = t_tab[:, (accb + b0) * F : (accb + b0 + bl) * F]
                    ew = t_ew[:, accw + b0 * m : accw + (b0 + bl) * m]
                    nc.vector.tensor_tensor(
                        t_out[:].rearrange("p (b f m) -> p b f m", f=F, m=m),
                        src.rearrange("p (b f) -> p b f", f=F)
                        .unsqueeze(3)
                        .broadcast_to([P, bl, F, m]),
                        ew.rearrange("p (b m) -> p b m", m=m)
                        .unsqueeze(2)
                        .broadcast_to([P, bl, F, m]),
                        MUL,
                    )
                    nc.sync.dma_start(
                        out=msgs[:, accg + b0 * F * m : accg + (b0 + bl) * F * m],
                        in_=t_out[:],
                    )
                accb += tbm
                accw += tbm * m
                accg += tbm * F * m
    legalize_waits(nc)
    return nc


def _reduce_classes(nc, tc, pool, apool, st, F, msgs_in):
    nb_all = st.ntot // P
    t_agg = apool.tile([P, nb_all * F], FP)
    accf = 0
    accb = 0
    for j in range(1, st.jmax + 1):
        nbj = int(st.nj[j]) // P
        L = 8 * j
        for b0, bl in _chunks(nbj, max(1, _CHUNK // (F * L))):
            t_in = pool.tile([P, bl * F * L], FP, tag="rin")
            nc.sync.dma_start(
                out=t_in[:],
                in_=msgs_in[:, accf + b0 * F * L : accf + (b0 + bl) * F * L],
            )
            nc.vector.tensor_reduce(
                out=t_agg[:, (accb + b0) * F : (accb + b0 + bl) * F],
                in_=t_in[:].rearrange("p (b f l) -> p b f l", f=F, l=L),
                axis=mybir.AxisListType.X,
                op=ADD,
            )
        accf += nbj * F * L
        accb += nbj
    return t_agg


def build_PC(st):
    """msgs_s + x_own + dis_own + weights -> ys [P, nb*F2] (col 7 zero)."""
    nc = bass.Bass("TRN2", num_devices=S)
    nb = st.ntot // P
    SF = sum((int(st.nj[j]) // P) * F1 * 8 * j for j in range(1, st.jmax + 1))
    msgs = nc.dram_tensor("msgs", (P, SF), FP, kind="ExternalInput")
    x_own = nc.dram_tensor("x_own", (P, nb * F1), FP, kind="ExternalInput")
    dis_own = nc.dram_tensor("dis_own", (P, nb), FP, kind="ExternalInput")
    w1b = nc.dram_tensor("w1b", (P, F1 * 16), FP, kind="ExternalInput")
    b1b = nc.dram_tensor("b1b", (P, 16), FP, kind="ExternalInput")
    w2b = nc.dram_tensor("w2b", (P, 16 * 7), FP, kind="ExternalInput")
    ys_o = nc.dram_tensor("ys", (P, nb * F2), FP, kind="ExternalOutput")
    with tile.TileContext(nc) as tc:
        with tc.tile_pool(name="sb", bufs=3) as pool, tc.tile_pool(
            name="acc", bufs=1
        ) as apool:
            t_agg = _reduce_classes(nc, tc, pool, apool, st, F1, msgs)
            t_xo = apool.tile([P, nb * F1], FP)
            nc.sync.dma_start(out=t_xo[:], in_=x_own[:])
            t_do = apool.tile([P, nb], FP)
            nc.sync.dma_start(out=t_do[:], in_=dis_own[:])
            t_w1 = apool.tile([P, F1 * 16], FP)
            nc.sync.dma_start(out=t_w1[:], in_=w1b[:])
            t_b1 = apool.tile([P, 16], FP)
            nc.sync.dma_start(out=t_b1[:], in_=b1b[:])
            t_w2 = apool.tile([P, 16 * 7], FP)
            nc.sync.dma_start(out=t_w2[:], in_=w2b[:])

            dis_b = t_do[:].unsqueeze(2).broadcast_to([P, nb, F1])
            agg_r = t_agg[:].rearrange("p (b f) -> p b f", f=F1)
            xo_r = t_xo[:].rearrange("p (b f) -> p b f", f=F1)
            # v = dis * (agg + dis * x_own)
            nc.vector.tensor_tensor(xo_r, xo_r, dis_b, MUL)
            nc.vector.tensor_tensor(agg_r, agg_r, xo_r, ADD)
            nc.vector.tensor_tensor(agg_r, agg_r, dis_b, MUL)
            # h = relu(v @ W1 + b1)   (v[...,3] is zero-padded; W1 row 3 = 0)
            t_h = apool.tile([P, nb * 16], FP)
            h_r = t_h[:].rearrange("p (b o) -> p b o", o=16)
            t_tmp = apool.tile([P, nb * 16], FP)
            tmp_r = t_tmp[:].rearrange("p (b o) -> p b o", o=16)
            for i in range(3):
                vi = (
                    t_agg[:]
                    .rearrange("p (b f) -> p b f", f=F1)[:, :, i : i + 1]
                    .broadcast_to([P, nb, 16])
                )
                wrow = (
                    t_w1[:, i * 16 : (i + 1) * 16]
                    .unsqueeze(1)
                    .broadcast_to([P, nb, 16])
                )
                if i == 0:
                    nc.vector.tensor_tensor(h_r, vi, wrow, MUL)
                else:
                    nc.vector.tensor_tensor(tmp_r, vi, wrow, MUL)
                    nc.vector.tensor_tensor(h_r, h_r, tmp_r, ADD)
            nc.vector.tensor_tensor(
                h_r, h_r, t_b1[:].unsqueeze(1).broadcast_to([P, nb, 16]), ADD
            )
            nc.vector.tensor_scalar(
                t_h[:], t_h[:], 0.0, None, mybir.AluOpType.max
            )
            # ys0 = h @ W2 ; ys = dis * ys0 ; pad col 7 with zeros
            t_ys = apool.tile([P, nb * F2], FP)
            nc.vector.memset(t_ys[:], 0.0)
            ys_r = t_ys[:].rearrange("p (b o) -> p b o", o=F2)[:, :, 0:7]
            t_t7 = apool.tile([P, nb * 7], FP)
            t7_r = t_t7[:].rearrange("p (b o) -> p b o", o=7)
            for k in range(16):
                hk = h_r[:, :, k : k + 1].broadcast_to([P, nb, 7])
                wrow = (
                    t_w2[:, k * 7 : (k + 1) * 7]
                    .unsqueeze(1)
                    .broadcast_to([P, nb, 7])
                )
                if k == 0:
                    nc.vector.tensor_tensor(ys_r, hk, wrow, MUL)
                else:
                    nc.vector.tensor_tensor(t7_r, hk, wrow, MUL)
                    nc.vector.tensor_tensor(ys_r, ys_r, t7_r, ADD)
            nc.vector.tensor_tensor(
                ys_r, ys_r, t_do[:].unsqueeze(2).broadcast_to([P, nb, 7]), MUL
            )
            nc.sync.dma_start(out=ys_o[:], in_=t_ys[:])
    legalize_waits(nc)
    return nc


def build_PE(st):
    """msgs2_s + ys_own + dis_own + b2 -> out2 [P, nb*F2]."""
    nc = bass.Bass("TRN2", num_devices=S)
    nb = st.ntot // P
    SF = sum((int(st.nj[j]) // P) * F2 * 8 * j for j in range(1, st.jmax + 1))
    msgs = nc.dram_tensor("msgs", (P, SF), FP, kind="ExternalInput")
    ys_own = nc.dram_tensor("ys_own", (P, nb * F2), FP, kind="ExternalInput")
    dis_own = nc.dram_tensor("dis_own", (P, nb), FP, kind="ExternalInput")
    b2b = nc.dram_tensor("b2b", (P, F2), FP, kind="ExternalInput")
    out_o = nc.dram_tensor("out", (P, nb * F2), FP, kind="ExternalOutput")
    with tile.TileContext(nc) as tc:
        with tc.tile_pool(name="sb", bufs=3) as pool, tc.tile_pool(
            name="acc", bufs=1
        ) as apool:
            t_agg = _reduce_classes(nc, tc, pool, apool, st, F2, msgs)
            t_yo = apool.tile([P, nb * F2], FP)
            nc.sync.dma_start(out=t_yo[:], in_=ys_own[:])
            t_do = apool.tile([P, nb], FP)
            nc.sync.dma_start(out=t_do[:], in_=dis_own[:])
            t_b2 = apool.tile([P, F2], FP)
            nc.sync.dma_start(out=t_b2[:], in_=b2b[:])
            agg_r = t_agg[:].rearrange("p (b f) -> p b f", f=F2)
            yo_r = t_yo[:].rearrange("p (b f) -> p b f", f=F2)
            dis_b = t_do[:].unsqueeze(2).broadcast_to([P, nb, F2])
            nc.vector.tensor_tensor(agg_r, agg_r, yo_r, ADD)
            nc.vector.tensor_tensor(agg_r, agg_r, dis_b, MUL)
            nc.vector.tensor_tensor(
                agg_r, agg_r, t_b2[:].unsqueeze(1).broadcast_to([P, nb, F2]), ADD
            )
            nc.sync.dma_start(out=out_o[:], in_=t_agg[:])
    legalize_waits(nc)
    return nc


# --------------------------------------------------------------------------
def kernel(x, edge_index, edge_weight, W1, b1, W2, b2):
    x = np.asarray(x, np.float32)
    ei = np.asarray(edge_index)
    ew = np.asarray(edge_weight, np.float32)
    W1 = np.asarray(W1, np.float32)
    b1 = np.asarray(b1, np.float32)
    W2 = np.asarray(W2, np.float32)
    b2 = np.asarray(b2, np.float32)

    PROGRAM_TIMES_NS.clear()
    st = build_structs(ei[0], ei[1], ew)
    nb = st.ntot // P
    RB = st.rtot // P
    tpp, tpb = tab_place(st)

    core_idx = []
    for c in range(S):
        cs = st.cores[c]
        gp, gfree, GF = gather_flat_index(st, cs, F1)
        sp, sfree, SF = scatter_flat_index(st, cs, F1)
        gp2, gfree2, GF2 = gather_flat_index(st, cs, F2)
        sp2, sfree2, SF2 = scatter_flat_index(st, cs, F2)
        _, sfree1, SF1 = scatter_flat_index(st, cs, 1)
        core_idx.append(
            (cs, gp, gfree, GF, sp, sfree, SF, gp2, gfree2, GF2, sp2, sfree2, SF2, sfree1, SF1)
        )

    # ---------------- P_A ----------------
    nc = build_PA(st)
    in_maps = []
    for c in range(S):
        cs = core_idx[c][0]
        sp = core_idx[c][4]
        sfree1 = core_idx[c][13]
        SF1 = core_idx[c][14]
        ews = np.zeros((P, SF1), np.float32)
        ews[sp, sfree1] = cs.eew
        in_maps.append({"ews": ews})
    res = _run(nc, in_maps, "PA_deg")
    dis_shard = [res[c]["dis"] for c in range(S)]

    dis_can = np.zeros(N, np.float32)
    owns = []
    for c in range(S):
        cs = core_idx[c][0]
        pown, bown = own_perm(st, cs)
        owns.append((pown, bown))
        dis_can[c * NS + np.arange(NS)] = dis_shard[c][pown, bown]

    # ---------------- P_B (expand L1) ----------------
    nc = build_expand(st, F1, scale_dis=True)
    in_maps = []
    for c in range(S):
        cs = core_idx[c][0]
        gp, gfree = core_idx[c][1], core_idx[c][2]
        x_tab = np.zeros((P, RB, F1), np.float32)
        dis_tab = np.zeros((P, RB), np.float32)
        valid = cs.tabrows >= 0
        rr = cs.tabrows[valid]
        x_tab[tpp[valid], tpb[valid], :3] = x[rr]
        dis_tab[tpp[valid], tpb[valid]] = dis_can[rr]
        EWT = sum((int(st.tm[m]) // P) * m for m in range(1, st.mmax + 1))
        ewg = np.zeros((P, EWT), np.float32)
        # ew slot (no feature axis): per class base/  b*m + occ
        base_w = np.zeros(st.mmax + 1, np.int64)
        accw = 0
        for m in range(1, st.mmax + 1):
            base_w[m] = accw
            accw += (int(st.tm[m]) // P) * m
        mm = cs.g_m
        q_local = cs.g_tabpos - st.base_tab[mm]
        tbm = st.tm[mm] // P
        wfree = base_w[mm] + (q_local % tbm) * mm + cs.g_occ
        ewg[q_local // tbm, wfree] = cs.eew
        in_maps.append(
            {
                "x_tab": x_tab.reshape(P, RB * F1),
                "dis_tab": dis_tab,
                "ewg": ewg,
            }
        )
        core_idx[c] = core_idx[c] + (wfree, ewg)
    res = _run(nc, in_maps, "PB_expand1")
    msgs_g = [res[c]["msgs"] for c in range(S)]

    # ---------------- host route L1 ----------------
    nc = build_PC(st)
    w1b = np.zeros((P, F1 * 16), np.float32)
    w1b[:, : 3 * 16] = np.broadcast_to(W1.reshape(1, 48), (P, 48))
    b1b = np.broadcast_to(b1.reshape(1, 16), (P, 16)).copy()
    w2b = np.broadcast_to(W2.reshape(1, 112), (P, 112)).copy()
    in_maps = []
    for c in range(S):
        cs = core_idx[c][0]
        gp, gfree = core_idx[c][1], core_idx[c][2]
        sp, sfree, SF = core_idx[c][4], core_idx[c][5], core_idx[c][6]
        msgs_s = np.zeros((P, SF), np.float32)
        jL = 8 * cs.jcls[cs.ecol]
        for fi in range(F1):
            msgs_s[sp, sfree + fi * jL] = msgs_g[c][gp, gfree + fi * cs.g_m]
        pown, bown = owns[c]
        x_own = np.zeros((P, nb, F1), np.float32)
        x_own[pown, bown, :3] = x[c * NS + np.arange(NS)]
        dis_own = np.zeros((P, nb), np.float32)
        dis_own[pown, bown] = dis_can[c * NS + np.arange(NS)]
        in_maps.append(
            {
                "msgs": msgs_s,
                "x_own": x_own.reshape(P, nb * F1),
                "dis_own": dis_own,
                "w1b": w1b,
                "b1b": b1b,
                "w2b": w2b,
            }
        )
        core_idx[c] = core_idx[c] + (dis_own,)
    res = _run(nc, in_maps, "PC_reduce1_mlp")
    ys_shard = [res[c]["ys"] for c in range(S)]

    ys_can = np.zeros((N, F2), np.float32)
    for c in range(S):
        pown, bown = owns[c]
        ys_can[c * NS + np.arange(NS)] = ys_shard[c].reshape(P, nb, F2)[pown, bown]

    # ---------------- P_D (expand L2) ----------------
    nc = build_expand(st, F2, scale_dis=False)
    in_maps = []
    for c in range(S):
        cs = core_idx[c][0]
        ewg = core_idx[c][16]
        ys_tab = np.zeros((P, RB, F2), np.float32)
        valid = cs.tabrows >= 0
        rr = cs.tabrows[valid]
        ys_tab[tpp[valid], tpb[valid]] = ys_can[rr]
        in_maps.append({"x_tab": ys_tab.reshape(P, RB * F2), "ewg": ewg})
    res = _run(nc, in_maps, "PD_expand2")
    msgs2_g = [res[c]["msgs"] for c in range(S)]

    # ---------------- host route L2 + P_E ----------------
    nc = build_PE(st)
    b2b = np.zeros((P, F2), np.float32)
    b2b[:, :] = b2
    in_maps = []
    for c in range(S):
        cs = core_idx[c][0]
        gp2, gfree2 = core_idx[c][7], core_idx[c][8]
        sp2, sfree2, SF2 = core_idx[c][10], core_idx[c][11], core_idx[c][12]
        msgs2_s = np.zeros((P, SF2), np.float32)
        jL = 8 * cs.jcls[cs.ecol]
        for fi in range(F2):
            msgs2_s[sp2, sfree2 + fi * jL] = msgs2_g[c][gp2, gfree2 + fi * cs.g_m]
        pown, bown = owns[c]
        ys_own = np.zeros((P, nb, F2), np.float32)
        ys_own[pown, bown] = ys_can[c * NS + np.arange(NS)]
        dis_own = core_idx[c][17]
        in_maps.append(
            {
                "msgs": msgs2_s,
                "ys_own": ys_own.reshape(P, nb * F2),
                "dis_own": dis_own,
                "b2b": b2b,
            }
        )
    res = _run(nc, in_maps, "PE_reduce2")

    out = np.zeros((N, 7), np.float32)
    for c in range(S):
        o = res[c]["out"].reshape(P, nb, F2)
        pown, bown = owns[c]
        out[c * NS + np.arange(NS)] = o[pown, bown, :7]
    return out


# revision 6
# speedup vs baseline: 1.0447x; 1.0447x over previous
"""GCN (2-layer) on 8 Trainium2 NeuronCores.

Strategy (graph/data parallel, per the node-range sharding hint):
- Nodes are sharded by range (25k per core); edges live on the core that
  owns their *destination* node; tiny weights are replicated.
- All irregular graph routing is converted on the host into REGULAR
  device-side layouts:
    * scatter side: destination nodes are bucketed by in-degree class
      j=ceil(d/8); each node gets exactly 8j message slots, so segment-sum
      becomes plain `tensor_reduce` over the innermost axis.
    * gather side: source rows are bucketed by multiplicity class m
      (# edges this core pulls from the row); the per-core permuted table
      is expanded by stride-0 broadcast copies [P,b,f] -> [P,b,f,m].
- The per-edge routing between gather order and scatter order (the
  "all-to-all on gathered messages") is staged through the host between
  device programs; every floating-point operation on values happens on
  device.

Five small SPMD programs: deg/dis -> expand L1 -> reduce L1 + MLP ->
expand L2 -> reduce L2.
"""
import sys

sys.path.insert(0, "/opt/trn_rl_repo")

import numpy as np

import bass_rust
from concourse import bass, mybir
from concourse.bass_utils import run_bass_kernel_spmd
import concourse.tile as tile

import os as _os

PROGRAM_TIMES_NS = []   # (name, exec_time_ns) per device program of last kernel() call


def _enable_tracing():
    import types
    import antenv
    if 'antenv.axon_hooks' in sys.modules:
        return True
    try:
        from trn_agent_boot.trn_boot import _ntff_profile_via_ctypes
        hook = _ntff_profile_via_ctypes('/opt/axon/libaxon_pjrt.so')
    except Exception:
        return False
    mod = types.ModuleType('antenv.axon_hooks')
    mod.get_axon_ntff_profile_hook = lambda: hook
    mod.set_axon_ntff_profile_hook = lambda h: None
    sys.modules['antenv.axon_hooks'] = mod
    antenv.axon_hooks = mod
    import concourse.bass_utils as _bu
    _bu.upload_artifacts = lambda tmpdir: f"local://{tmpdir}"
    return True


def _run(nc, in_maps, name):
    trace = bool(_os.environ.get('GCN_TRACE')) and _enable_tracing()
    r = run_bass_kernel_spmd(nc, in_maps, core_ids=CORE_IDS, trace=trace)
    if trace:
        PROGRAM_TIMES_NS.append((name, r.exec_time_ns))
    return r.results

S = 8
N = 200000
NS = N // S
P = 128
F1 = 4
F2 = 7
CORE_IDS = list(range(S))
FP = mybir.dt.float32
MUL = mybir.AluOpType.mult
ADD = mybir.AluOpType.add


def _ceil(a, b):
    return -(-a // b)


# --------------------------------------------------------------------------
# walrus on this toolchain accepts at most ONE sync-wait per instruction;
# Tile emits several at DAG joins / kernel-tail drain. Hoist excess waits
# onto fresh same-engine NoOps inserted immediately before the violator.
def legalize_waits(nc):
    nop_idx = 0
    for f in nc.m.functions:
        for bb in f.blocks:
            il = bb.instructions
            if not any(
                inst.sync_info is not None
                and len(inst.sync_info.on_wait or []) > 1
                for inst in il
            ):
                continue
            new_il = []
            for inst in il:
                si = inst.sync_info
                w = list(si.on_wait or []) if si is not None else []
                if len(w) > 1:
                    for extra in w[:-1]:
                        nop = mybir.InstNoOp(
                            name=f"I-waitsplit-{nop_idx}", ins=[], outs=[]
                        )
                        nop_idx += 1
                        nop.engine = inst.engine
                        nop.sync_info = bass_rust.SyncInfo(
                            on_wait=[extra], on_update=[]
                        )
                        new_il.append(nop)
                    si.on_wait = [w[-1]]
                new_il.append(inst)
            bb.instructions = new_il


# --------------------------------------------------------------------------
# host-side structure building
class _O:
    pass


def build_structs(row, col, ew):
    row = row.astype(np.int64)
    col = col.astype(np.int64)
    cores = []
    for c in range(S):
        cs = _O()
        m = (col // NS) == c
        cs.erow = row[m]
        cs.ecol = (col[m] - c * NS).astype(np.int64)
        cs.eew = ew[m].astype(np.float32)
        cores.append(cs)

    for cs in cores:
        d = np.bincount(cs.ecol, minlength=NS)
        cs.indeg = d
        cs.jcls = np.maximum(1, _ceil(np.maximum(d, 1), 8))
    jmax = max(int(cs.jcls.max()) for cs in cores)
    nj = np.zeros(jmax + 1, np.int64)
    for j in range(1, jmax + 1):
        njc = max(int((cs.jcls == j).sum()) for cs in cores)
        nj[j] = _ceil(max(njc, 1), P) * P
    for cs in cores:
        nodepos = np.full(NS, -1, np.int64)
        pos = 0
        for j in range(1, jmax + 1):
            nodes = np.nonzero(cs.jcls == j)[0]
            nodepos[nodes] = pos + np.arange(len(nodes))
            pos += nj[j]
        cs.nodepos = nodepos
    ntot = int(nj[1:].sum())

    for cs in cores:
        rows_used, inv, cnt = np.unique(
            cs.erow, return_inverse=True, return_counts=True
        )
        cs.g_rows = rows_used
        cs.g_cnt = cnt
        cs.g_inv = inv
    mmax = max(int(cs.g_cnt.max()) for cs in cores)
    tm = np.zeros(mmax + 1, np.int64)
    for mcl in range(1, mmax + 1):
        tmc = max(int((cs.g_cnt == mcl).sum()) for cs in cores)
        tm[mcl] = _ceil(max(tmc, 1), P) * P
    for cs in cores:
        tabrows = np.full(int(tm[1:].sum()), -1, np.int64)
        pos_of_unique = np.empty(len(cs.g_rows), np.int64)
        pos = 0
        for mcl in range(1, mmax + 1):
            sel = cs.g_cnt == mcl
            rr = cs.g_rows[sel]
            tabrows[pos : pos + len(rr)] = rr
            pos_of_unique[sel] = pos - 0 + np.arange(len(rr))
            pos += tm[mcl]
        cs.tabrows = tabrows
        cs.g_tabpos = pos_of_unique[cs.g_inv]
    rtot = int(tm[1:].sum())

    st = _O()
    st.jmax, st.nj, st.ntot = jmax, nj, ntot
    st.mmax, st.tm, st.rtot = mmax, tm, rtot
    st.cores = cores
    base_tab = np.zeros(mmax + 2, np.int64)
    base_tab[1:] = np.cumsum(tm)[: mmax + 1]
    st.base_tab = base_tab

    for cs in cores:
        mm = cs.g_cnt[cs.g_inv]
        ord_ = np.argsort(cs.g_inv, kind="stable")
        inv_sorted = cs.g_inv[ord_]
        first = np.r_[True, inv_sorted[1:] != inv_sorted[:-1]]
        idx_of_first = np.maximum.accumulate(
            np.where(first, np.arange(len(ord_)), 0)
        )
        occ = np.empty(len(cs.erow), np.int64)
        occ[ord_] = np.arange(len(ord_)) - idx_of_first
        cs.g_m = mm
        cs.g_occ = occ
    return st


def gather_flat_index(st, cs, f):
    mm = cs.g_m
    q_local = cs.g_tabpos - st.base_tab[mm]
    tbm = st.tm[mm] // P
    p = q_local // tbm
    b = q_local % tbm
    base_free = np.zeros(st.mmax + 1, np.int64)
    acc = 0
    for mcl in range(1, st.mmax + 1):
        base_free[mcl] = acc
        acc += (st.tm[mcl] // P) * f * mcl
    gfree = base_free[mm] + b * (f * mm) + cs.g_occ
    return p, gfree, acc


def scatter_flat_index(st, cs, f):
    nodes = cs.ecol
    j = cs.jcls[nodes]
    q = cs.nodepos[nodes]
    base_node = np.zeros(st.jmax + 1, np.int64)
    accn = 0
    for jj in range(1, st.jmax + 1):
        base_node[jj] = accn
        accn += st.nj[jj]
    q_local = q - base_node[j]
    nbj = st.nj[j] // P
    p = q_local // nbj
    b = q_local % nbj
    base_free = np.zeros(st.jmax + 1, np.int64)
    acc = 0
    for jj in range(1, st.jmax + 1):
        base_free[jj] = acc
        acc += (st.nj[jj] // P) * f * 8 * jj
    ord_ = np.argsort(nodes, kind="stable")
    ns = nodes[ord_]
    first = np.r_[True, ns[1:] != ns[:-1]]
    idx_of_first = np.maximum.accumulate(np.where(first, np.arange(len(ord_)), 0))
    k = np.empty(len(nodes), np.int64)
    k[ord_] = np.arange(len(nodes)) - idx_of_first
    sfree = base_free[j] + b * (f * 8 * j) + k
    return p, sfree, acc


def own_perm(st, cs):
    """per local node: (partition, block) in the scatter/agg [P, ntot/P] grid"""
    base_node = np.zeros(st.jmax + 1, np.int64)
    accn = 0
    for jj in range(1, st.jmax + 1):
        base_node[jj] = accn
        accn += st.nj[jj]
    base_nb = np.zeros(st.jmax + 1, np.int64)
    accb = 0
    for jj in range(1, st.jmax + 1):
        base_nb[jj] = accb
        accb += st.nj[jj] // P
    j = cs.jcls
    ql = cs.nodepos - base_node[j]
    nbj = st.nj[j] // P
    return ql // nbj, base_nb[j] + ql % nbj


def tab_place(st):
    """table position q -> (p, block) in [P, rtot/P]."""
    pp = np.empty(st.rtot, np.int64)
    bb = np.empty(st.rtot, np.int64)
    accb = 0
    pos = 0
    for mcl in range(1, st.mmax + 1):
        tbm = st.tm[mcl] // P
        ql = np.arange(st.tm[mcl])
        pp[pos : pos + st.tm[mcl]] = ql // tbm
        bb[pos : pos + st.tm[mcl]] = accb + ql % tbm
        pos += st.tm[mcl]
        accb += tbm
    return pp, bb


# --------------------------------------------------------------------------
# device programs
_CHUNK = 6144  # free-size chunk budget (fp32 elems per partition) for streaming


def _chunks(total, step):
    out = []
    o = 0
    while o < total:
        out.append((o, min(step, total - o)))
        o += step
    return out


def build_PA(st):
    """ews [P, SF1] -> dis [P, ntot/P] ; deg = reduce + 1 ; dis = rsqrt."""
    nc = bass.Bass("TRN2", num_devices=S)
    SF1 = sum((int(st.nj[j]) // P) * 8 * j for j in range(1, st.jmax + 1))
    nb_all = st.ntot // P
    ews = nc.dram_tensor("ews", (P, SF1), FP, kind="ExternalInput")
    dis_o = nc.dram_tensor("dis", (P, nb_all), FP, kind="ExternalOutput")
    with tile.TileContext(nc) as tc:
        with tc.tile_pool(name="sb", bufs=2) as pool, tc.tile_pool(
            name="acc", bufs=1
        ) as apool:
            t_deg = apool.tile([P, nb_all], FP)
            accf = 0
            accb = 0
            for j in range(1, st.jmax + 1):
                nbj = int(st.nj[j]) // P
                L = 8 * j
                for b0, bl in _chunks(nbj, max(1, _CHUNK // L)):
                    t_in = pool.tile([P, bl * L], FP, tag="in")
                    nc.sync.dma_start(
                        out=t_in[:],
                        in_=ews[:, accf + b0 * L : accf + (b0 + bl) * L],
                    )
                    nc.vector.tensor_reduce(
                        out=t_deg[:, accb + b0 : accb + b0 + bl],
                        in_=t_in[:].rearrange("p (b l) -> p b l", l=L),
                        axis=mybir.AxisListType.X,
                        op=ADD,
                    )
                accf += nbj * L
                accb += nbj
            t_d1 = apool.tile([P, nb_all], FP)
            nc.vector.tensor_scalar_add(t_d1[:], t_deg[:], 1.0)
            t_sq = apool.tile([P, nb_all], FP)
            nc.scalar.sqrt(t_sq[:], t_d1[:])
            t_r = apool.tile([P, nb_all], FP)
            nc.vector.reciprocal(t_r[:], t_sq[:])
            # one Newton step: y <- y * (1.5 - 0.5 * d * y^2)
            t_y2 = apool.tile([P, nb_all], FP)
            nc.vector.tensor_tensor(t_y2[:], t_r[:], t_r[:], MUL)
            nc.vector.tensor_tensor(t_y2[:], t_y2[:], t_d1[:], MUL)
            nc.vector.tensor_scalar_mul(t_y2[:], t_y2[:], -0.5)
            nc.vector.tensor_scalar_add(t_y2[:], t_y2[:], 1.5)
            nc.vector.tensor_tensor(t_r[:], t_r[:], t_y2[:], MUL)
            nc.sync.dma_start(out=dis_o[:], in_=t_r[:])
    legalize_waits(nc)
    return nc


def build_expand(st, F, scale_dis):
    """x_tab [P, RB*F] (+ dis_tab [P, RB] if scale_dis), ewg [P, EWT]
    -> msgs_g [P, GF]."""
    nc = bass.Bass("TRN2", num_devices=S)
    RB = st.rtot // P
    EWT = sum((int(st.tm[m]) // P) * m for m in range(1, st.mmax + 1))
    GF = sum((int(st.tm[m]) // P) * F * m for m in range(1, st.mmax + 1))
    x_tab = nc.dram_tensor("x_tab", (P, RB * F), FP, kind="ExternalInput")
    ewg = nc.dram_tensor("ewg", (P, EWT), FP, kind="ExternalInput")
    if scale_dis:
        dis_tab = nc.dram_tensor("dis_tab", (P, RB), FP, kind="ExternalInput")
    msgs = nc.dram_tensor("msgs", (P, GF), FP, kind="ExternalOutput")
    with tile.TileContext(nc) as tc:
        with tc.tile_pool(name="tab", bufs=1) as tpool, tc.tile_pool(
            name="str", bufs=3
        ) as pool:
            t_tab = tpool.tile([P, RB * F], FP)
            nc.sync.dma_start(out=t_tab[:], in_=x_tab[:])
            if scale_dis:
                t_dis = tpool.tile([P, RB], FP)
                nc.sync.dma_start(out=t_dis[:], in_=dis_tab[:])
                nc.vector.tensor_tensor(
                    t_tab[:].rearrange("p (b f) -> p b f", f=F),
                    t_tab[:].rearrange("p (b f) -> p b f", f=F),
                    t_dis[:].unsqueeze(2).broadcast_to([P, RB, F]),
                    MUL,
                )
            t_ew = tpool.tile([P, EWT], FP)
            nc.sync.dma_start(out=t_ew[:], in_=ewg[:])
            accb = 0
            accw = 0
            accg = 0
            for m in range(1, st.mmax + 1):
                tbm = int(st.tm[m]) // P
                for b0, bl in _chunks(tbm, max(1, _CHUNK // (F * m))):
                    t_out = pool.tile([P, bl * F * m], FP, tag="out")
                    src = t_tab[:, (accb + b0) * F : (accb + b0 + bl) * F]
                    ew = t_ew[:, accw + b0 * m : accw + (b0 + bl) * m]
                    nc.vector.tensor_tensor(
                        t_out[:].rearrange("p (b f m) -> p b f m", f=F, m=m),
                        src.rearrange("p (b f) -> p b f", f=F)
                        .unsqueeze(3)
                        .broadcast_to([P, bl, F, m]),
                        ew.rearrange("p (b m) -> p b m", m=m)
                        .unsqueeze(2)
                        .broadcast_to([P, bl, F, m]),
                        MUL,
                    )
                    nc.sync.dma_start(
                        out=msgs[:, accg + b0 * F * m : accg + (b0 + bl) * F * m],
                        in_=t_out[:],
                    )
                accb += tbm
                accw += tbm * m
                accg += tbm * F * m
    legalize_waits(nc)
    return nc


def _reduce_classes(nc, tc, pool, apool, st, F, msgs_in):
    nb_all = st.ntot // P
    t_agg = apool.tile([P, nb_all * F], FP)
    accf = 0
    accb = 0
    for j in range(1, st.jmax + 1):
        nbj = int(st.nj[j]) // P
        L = 8 * j
        for b0, bl in _chunks(nbj, max(1, _CHUNK // (F * L))):
            t_in = pool.tile([P, bl * F * L], FP, tag="rin")
            nc.sync.dma_start(
                out=t_in[:],
                in_=msgs_in[:, accf + b0 * F * L : accf + (b0 + bl) * F * L],
            )
            nc.vector.tensor_reduce(
                out=t_agg[:, (accb + b0) * F : (accb + b0 + bl) * F],
                in_=t_in[:].rearrange("p (b f l) -> p b f l", f=F, l=L),
                axis=mybir.AxisListType.X,
                op=ADD,
            )
        accf += nbj * F * L
        accb += nbj
    return t_agg


def build_PC(st):
    """msgs_s + x_own + dis_own + weights -> ys [P, nb*F2] (col 7 zero)."""
    nc = bass.Bass("TRN2", num_devices=S)
    nb = st.ntot // P
    SF = sum((int(st.nj[j]) // P) * F1 * 8 * j for j in range(1, st.jmax + 1))
    msgs = nc.dram_tensor("msgs", (P, SF), FP, kind="ExternalInput")
    x_own = nc.dram_tensor("x_own", (P, nb * F1), FP, kind="ExternalInput")
    dis_own = nc.dram_tensor("dis_own", (P, nb), FP, kind="ExternalInput")
    w1b = nc.dram_tensor("w1b", (P, F1 * 16), FP, kind="ExternalInput")
    b1b = nc.dram_tensor("b1b", (P, 16), FP, kind="ExternalInput")
    w2b = nc.dram_tensor("w2b", (P, 16 * 7), FP, kind="ExternalInput")
    ys_o = nc.dram_tensor("ys", (P, nb * F2), FP, kind="ExternalOutput")
    with tile.TileContext(nc) as tc:
        with tc.tile_pool(name="sb", bufs=3) as pool, tc.tile_pool(
            name="acc", bufs=1
        ) as apool:
            t_agg = _reduce_classes(nc, tc, pool, apool, st, F1, msgs)
            t_xo = apool.tile([P, nb * F1], FP)
            nc.sync.dma_start(out=t_xo[:], in_=x_own[:])
            t_do = apool.tile([P, nb], FP)
            nc.sync.dma_start(out=t_do[:], in_=dis_own[:])
            t_w1 = apool.tile([P, F1 * 16], FP)
            nc.sync.dma_start(out=t_w1[:], in_=w1b[:])
            t_b1 = apool.tile([P, 16], FP)
            nc.sync.dma_start(out=t_b1[:], in_=b1b[:])
            t_w2 = apool.tile([P, 16 * 7], FP)
            nc.sync.dma_start(out=t_w2[:], in_=w2b[:])

            dis_b = t_do[:].unsqueeze(2).broadcast_to([P, nb, F1])
            agg_r = t_agg[:].rearrange("p (b f) -> p b f", f=F1)
            xo_r = t_xo[:].rearrange("p (b f) -> p b f", f=F1)
            # v = dis * (agg + dis * x_own)
            nc.vector.tensor_tensor(xo_r, xo_r, dis_b, MUL)
            nc.vector.tensor_tensor(agg_r, agg_r, xo_r, ADD)
            nc.vector.tensor_tensor(agg_r, agg_r, dis_b, MUL)
            # h = relu(v @ W1 + b1)   (v[...,3] is zero-padded; W1 row 3 = 0)
            t_h = apool.tile([P, nb * 16], FP)
            h_r = t_h[:].rearrange("p (b o) -> p b o", o=16)
            t_tmp = apool.tile([P, nb * 16], FP)
            tmp_r = t_tmp[:].rearrange("p (b o) -> p b o", o=16)
            for i in range(3):
                vi = (
                    t_agg[:]
                    .rearrange("p (b f) -> p b f", f=F1)[:, :, i : i + 1]
                    .broadcast_to([P, nb, 16])
                )
                wrow = (
                    t_w1[:, i * 16 : (i + 1) * 16]
                    .unsqueeze(1)
                    .broadcast_to([P, nb, 16])
                )
                if i == 0:
                    nc.vector.tensor_tensor(h_r, vi, wrow, MUL)
                else:
                    nc.vector.tensor_tensor(tmp_r, vi, wrow, MUL)
                    nc.vector.tensor_tensor(h_r, h_r, tmp_r, ADD)
            nc.vector.tensor_tensor(
                h_r, h_r, t_b1[:].unsqueeze(1).broadcast_to([P, nb, 16]), ADD
            )
            nc.vector.tensor_scalar(
                t_h[:], t_h[:], 0.0, None, mybir.AluOpType.max
            )
            # ys0 = h @ W2 ; ys = dis * ys0 ; pad col 7 with zeros
            t_ys = apool.tile([P, nb * F2], FP)
            nc.vector.memset(t_ys[:], 0.0)
            ys_r = t_ys[:].rearrange("p (b o) -> p b o", o=F2)[:, :, 0:7]
            t_t7 = apool.tile([P, nb * 7], FP)
            t7_r = t_t7[:].rearrange("p (b o) -> p b o", o=7)
            for k in range(16):
                hk = h_r[:, :, k : k + 1].broadcast_to([P, nb, 7])
                wrow = (
                    t_w2[:, k * 7 : (k + 1) * 7]
                    .unsqueeze(1)
                    .broadcast_to([P, nb, 7])
                )
                if k == 0:
                    nc.vector.tensor_tensor(ys_r, hk, wrow, MUL)
                else:
                    nc.vector.tensor_tensor(t7_r, hk, wrow, MUL)
                    nc.vector.tensor_tensor(ys_r, ys_r, t7_r, ADD)
            nc.vector.tensor_tensor(
                ys_r, ys_r, t_do[:].unsqueeze(2).broadcast_to([P, nb, 7]), MUL
            )
            nc.sync.dma_start(out=ys_o[:], in_=t_ys[:])
    legalize_waits(nc)
    return nc


def build_PE(st):
    """msgs2_s + ys_own + dis_own + b2 -> out2 [P, nb*F2]."""
    nc = bass.Bass("TRN2", num_devices=S)
    nb = st.ntot // P
    SF = sum((int(st.nj[j]) // P) * F2 * 8 * j for j in range(1, st.jmax + 1))
    msgs = nc.dram_tensor("msgs", (P, SF), FP, kind="ExternalInput")
    ys_own = nc.dram_tensor("ys_own", (P, nb * F2), FP, kind="ExternalInput")
    dis_own = nc.dram_tensor("dis_own", (P, nb), FP, kind="ExternalInput")
    b2b = nc.dram_tensor("b2b", (P, F2), FP, kind="ExternalInput")
    out_o = nc.dram_tensor("out", (P, nb * F2), FP, kind="ExternalOutput")
    with tile.TileContext(nc) as tc:
        with tc.tile_pool(name="sb", bufs=3) as pool, tc.tile_pool(
            name="acc", bufs=1
        ) as apool:
            t_agg = _reduce_classes(nc, tc, pool, apool, st, F2, msgs)
            t_yo = apool.tile([P, nb * F2], FP)
            nc.sync.dma_start(out=t_yo[:], in_=ys_own[:])
            t_do = apool.tile([P, nb], FP)
            nc.sync.dma_start(out=t_do[:], in_=dis_own[:])
            t_b2 = apool.tile([P, F2], FP)
            nc.sync.dma_start(out=t_b2[:], in_=b2b[:])
            agg_r = t_agg[:].rearrange("p (b f) -> p b f", f=F2)
            yo_r = t_yo[:].rearrange("p (b f) -> p b f", f=F2)
            dis_b = t_do[:].unsqueeze(2).broadcast_to([P, nb, F2])
            nc.vector.tensor_tensor(agg_r, agg_r, yo_r, ADD)
            nc.vector.tensor_tensor(agg_r, agg_r, dis_b, MUL)
            nc.vector.tensor_tensor(
                agg_r, agg_r, t_b2[:].unsqueeze(1).broadcast_to([P, nb, F2]), ADD
            )
            nc.sync.dma_start(out=out_o[:], in_=t_agg[:])
    legalize_waits(nc)
    return nc


# --------------------------------------------------------------------------
def kernel(x, edge_index, edge_weight, W1, b1, W2, b2):
    x = np.asarray(x, np.float32)
    ei = np.asarray(edge_index)
    ew = np.asarray(edge_weight, np.float32)
    W1 = np.asarray(W1, np.float32)
    b1 = np.asarray(b1, np.float32)
    W2 = np.asarray(W2, np.float32)
    b2 = np.asarray(b2, np.float32)

    PROGRAM_TIMES_NS.clear()
    st = build_structs(ei[0], ei[1], ew)
    nb = st.ntot // P
    RB = st.rtot // P
    tpp, tpb = tab_place(st)

    core_idx = []
    for c in range(S):
        cs = st.cores[c]
        gp, gfree, GF = gather_flat_index(st, cs, F1)
        sp, sfree, SF = scatter_flat_index(st, cs, F1)
        gp2, gfree2, GF2 = gather_flat_index(st, cs, F2)
        sp2, sfree2, SF2 = scatter_flat_index(st, cs, F2)
        _, sfree1, SF1 = scatter_flat_index(st, cs, 1)
        core_idx.append(
            (cs, gp, gfree, GF, sp, sfree, SF, gp2, gfree2, GF2, sp2, sfree2, SF2, sfree1, SF1)
        )

    # ---------------- P_A ----------------
    nc = build_PA(st)
    in_maps = []
    for c in range(S):
        cs = core_idx[c][0]
        sp = core_idx[c][4]
        sfree1 = core_idx[c][13]
        SF1 = core_idx[c][14]
        ews = np.zeros((P, SF1), np.float32)
        ews[sp, sfree1] = cs.eew
        in_maps.append({"ews": ews})
    res = _run(nc, in_maps, "PA_deg")
    dis_shard = [res[c]["dis"] for c in range(S)]

    dis_can = np.zeros(N, np.float32)
    owns = []
    for c in range(S):
        cs = core_idx[c][0]
        pown, bown = own_perm(st, cs)
        owns.append((pown, bown))
        dis_can[c * NS + np.arange(NS)] = dis_shard[c][pown, bown]

    # ---------------- P_B (expand L1) ----------------
    nc = build_expand(st, F1, scale_dis=True)
    in_maps = []
    for c in range(S):
        cs = core_idx[c][0]
        gp, gfree = core_idx[c][1], core_idx[c][2]
        x_tab = np.zeros((P, RB, F1), np.float32)
        dis_tab = np.zeros((P, RB), np.float32)
        valid = cs.tabrows >= 0
        rr = cs.tabrows[valid]
        x_tab[tpp[valid], tpb[valid], :3] = x[rr]
        dis_tab[tpp[valid], tpb[valid]] = dis_can[rr]
        EWT = sum((int(st.tm[m]) // P) * m for m in range(1, st.mmax + 1))
        ewg = np.zeros((P, EWT), np.float32)
        # ew slot (no feature axis): per class base/  b*m + occ
        base_w = np.zeros(st.mmax + 1, np.int64)
        accw = 0
        for m in range(1, st.mmax + 1):
            base_w[m] = accw
            accw += (int(st.tm[m]) // P) * m
        mm = cs.g_m
        q_local = cs.g_tabpos - st.base_tab[mm]
        tbm = st.tm[mm] // P
        wfree = base_w[mm] + (q_local % tbm) * mm + cs.g_occ
        ewg[q_local // tbm, wfree] = cs.eew
        in_maps.append(
            {
                "x_tab": x_tab.reshape(P, RB * F1),
                "dis_tab": dis_tab,
                "ewg": ewg,
            }
        )
        core_idx[c] = core_idx[c] + (wfree, ewg)
    res = _run(nc, in_maps, "PB_expand1")
    msgs_g = [res[c]["msgs"] for c in range(S)]

    # ---------------- host route L1 ----------------
    nc = build_PC(st)
    w1b = np.zeros((P, F1 * 16), np.float32)
    w1b[:, : 3 * 16] = np.broadcast_to(W1.reshape(1, 48), (P, 48))
    b1b = np.broadcast_to(b1.reshape(1, 16), (P, 16)).copy()
    w2b = np.broadcast_to(W2.reshape(1, 112), (P, 112)).copy()
    in_maps = []
    for c in range(S):
        cs = core_idx[c][0]
        gp, gfree = core_idx[c][1], core_idx[c][2]
        sp, sfree, SF = core_idx[c][4], core_idx[c][5], core_idx[c][6]
        msgs_s = np.zeros((P, SF), np.float32)
        jL = 8 * cs.jcls[cs.ecol]
        for fi in range(F1):
            msgs_s[sp, sfree + fi * jL] = msgs_g[c][gp, gfree + fi * cs.g_m]
        pown, bown = owns[c]
        x_own = np.zeros((P, nb, F1), np.float32)
        x_own[pown, bown, :3] = x[c * NS + np.arange(NS)]
        dis_own = np.zeros((P, nb), np.float32)
        dis_own[pown, bown] = dis_can[c * NS + np.arange(NS)]
        in_maps.append(
            {
                "msgs": msgs_s,
                "x_own": x_own.reshape(P, nb * F1),
                "dis_own": dis_own,
                "w1b": w1b,
                "b1b": b1b,
                "w2b": w2b,
            }
        )
        core_idx[c] = core_idx[c] + (dis_own,)
    res = _run(nc, in_maps, "PC_reduce1_mlp")
    ys_shard = [res[c]["ys"] for c in range(S)]

    ys_can = np.zeros((N, F2), np.float32)
    for c in range(S):
        pown, bown = owns[c]
        ys_can[c * NS + np.arange(NS)] = ys_shard[c].reshape(P, nb, F2)[pown, bown]

    # ---------------- P_D (expand L2) ----------------
    nc = build_expand(st, F2, scale_dis=False)
    in_maps = []
    for c in range(S):
        cs = core_idx[c][0]
        ewg = core_idx[c][16]
        ys_tab = np.zeros((P, RB, F2), np.float32)
        valid = cs.tabrows >= 0
        rr = cs.tabrows[valid]
        ys_tab[tpp[valid], tpb[valid]] = ys_can[rr]
        in_maps.append({"x_tab": ys_tab.reshape(P, RB * F2), "ewg": ewg})
    res = _run(nc, in_maps, "PD_expand2")
    msgs2_g = [res[c]["msgs"] for c in range(S)]

    # ---------------- host route L2 + P_E ----------------
    nc = build_PE(st)
    b2b = np.zeros((P, F2), np.float32)
    b2b[:, :] = b2
    in_maps = []
    for c in range(S):
        cs = core_idx[c][0]
        gp2, gfree2 = core_idx[c][7], core_idx[c][8]
        sp2, sfree2, SF2 = core_idx[c][10], core_idx[c][11], core_idx[c][12]
        msgs2_s = np.zeros((P, SF2), np.float32)
        jL = 8 * cs.jcls[cs.ecol]
        for fi in range(F2):
            msgs2_s[sp2, sfree2 + fi * jL] = msgs2_g[c][gp2, gfree2 + fi * cs.g_m]
        pown, bown = owns[c]
        ys_own = np.zeros((P, nb, F2), np.float32)
        ys_own[pown, bown] = ys_can[c * NS + np.arange(NS)]
        dis_own = core_idx[c][17]
        in_maps.append(
            {
                "msgs": msgs2_s,
                "ys_own": ys_own.reshape(P, nb * F2),
                "dis_own": dis_own,
                "b2b": b2b,
            }
        )
    res = _run(nc, in_maps, "PE_reduce2")

    out = np.zeros((N, 7), np.float32)
    for c in range(S):
        o = res[c]["out"].reshape(P, nb, F2)
        pown, bown = owns[c]
        out[c * NS + np.arange(NS)] = o[pown, bown, :7]
    return out
